# revision 21
# baseline (speedup 1.0000x reference)
"""Trainium2 Bass kernel for nn_CharacterEmbeddingLayer.

Computation (see reference):
  embed = char_vectors[char_idxs]                       # [B,S,16,64]
  per window w in (2,3,4,5):
      h_w = max_l tanh(conv_w(embed))                   # [B,S,100]
  x = concat(h_w) @ w_proj.T                            # [B,S,128]
  2x highway: x = g*relu(Wt x+bt) + (1-g)*x, g=sigmoid(Wg x+bg)

Device mapping (per core, data-parallel over batch: 8 rows => 3200 tokens):
  - one-hot built on DVE (tensor_scalar is_equal, 4x mode: int16 in,
    bf16 out) from
    broadcast-DMA'd indices vs a per-partition iota column
  - embeddings materialized in "paired" layout E0[128=(dim, char-parity), 8, T]
    via PE matmul char_vectors.T @ one-hot (even chars -> partitions 0:64,
    odd chars -> partitions 64:128) so conv contractions get K=128
  - conv (bf16): per (window, position) accumulated matmuls with host-prestacked
    filter tensors, positions grouped 3-to-a-PSUM-tile; tanh deferred past the
    max-pool (monotonic)
  - max-pool split: ACT batch-extracts some groups PSUM->SBUF bf16, DVE
    grouped-reduces the others straight from PSUM, then a bf16 tensor_max tree
  - projection + highway on PE/ACT/DVE; output stored feature-major f32 and
    transposed on the host.
"""

import sys

sys.path.insert(0, "/opt/trn_rl_repo")

import numpy as np
import ml_dtypes

B, S, W, D = 64, 400, 16, 64
VOCAB, HID, NF = 96, 128, 100
WINDOWS = (2, 3, 4, 5)
N_CORES = 8
TOK_PER_CORE = B * S // N_CORES  # 3200
T = 512  # max tokens per tile (PSUM bank = 512 fp32)
TILES = [(t0, min(T, TOK_PER_CORE - t0)) for t0 in range(0, TOK_PER_CORE, T)]
N_TILES = len(TILES)  # 6x512 + 1x128
GRP = 3  # conv positions per PSUM group tile (3 banks)

_cache = {}

BF16 = ml_dtypes.bfloat16


# ---------------------------------------------------------------- schedule
def build_schedule():
    """Conv decomposition into K<=128 matmul units against the paired E layout.

    units: dict key -> index; key = ('pair', w, j) => stacked [F_j; F_{j+1}],
           ('single', w, j) => F_j in its fixed parity half, other half zero.
    sched: list of (w, [ops per l]); op = (unit_idx, pair_index). All matmuls
           are full K=128 against e_sb[:, pair, :].
    """
    units = {}

    def uidx(key):
        if key not in units:
            units[key] = len(units)
        return units[key]

    sched = []
    for w in WINDOWS:
        L = W - w + 1
        wl = []
        for l in range(L):
            ops = []
            j = 0
            while j < w:
                c = l + j
                if c % 2 == 0 and j + 1 < w:
                    ops.append((uidx(("pair", w, j)), c // 2))
                    j += 2
                else:
                    ops.append((uidx(("single", w, j)), c // 2))
                    j += 1
            wl.append(ops)
        sched.append((w, wl))
    return units, sched


def window_groups(L):
    """Split L positions into groups of <=GRP, alternating A(CT)/D(VE)."""
    out = []
    l = 0
    gi = 0
    while l < L:
        n = min(GRP, L - l)
        out.append((l, n))
        l += n
        gi += 1
    return out


# ---------------------------------------------------------------- host prep
def single_half(w, j):
    """Parity half a ('single', w, j) unit reads: j=0 singles only happen at
    odd l (char parity 1); trailing singles land on even chars (half 0)."""
    return 1 if j == 0 else 0


def prep_weights(char_vectors, filts, w_proj, hw_ws, hw_bs):
    """Build the DRAM-side packed weight arrays (all tiny)."""
    units, _ = build_schedule()
    U = len(units)
    wconv = np.zeros((128, U, 128), np.float32)
    for (kind, w, j), u in units.items():
        f = filts[w].reshape(NF, w, D)  # [100, w, 64]
        fj = f[:, j, :].T  # [64, 100] = lhsT block
        if kind == "pair":
            fj1 = f[:, j + 1, :].T
            wconv[0:64, u, 0:NF] = fj
            wconv[64:128, u, 0:NF] = fj1
        else:
            # zero-padded half: two concurrent K=64 row-tiles in one
            # accumulation group lock up the device, so singles run as
            # plain K=128 matmuls with zeros in the unused half
            half = single_half(w, j)
            wconv[64 * half:64 * half + 64, u, 0:NF] = fj
    wproj = np.zeros((128, 4, 128), np.float32)
    for c in range(4):
        wproj[0:NF, c, :] = w_proj[:, c * NF:(c + 1) * NF].T
    whw = np.zeros((128, 4, 128), np.float32)
    for i, wm in enumerate(hw_ws):  # [t_w0, g_w0, t_w1, g_w1]
        whw[:, i, :] = wm.T
    bias = np.zeros((128, 4), np.float32)
    for i, bv in enumerate(hw_bs):  # [t_b0, g_b0, t_b1, g_b1]
        bias[:, i] = bv
    return {
        "cv": np.ascontiguousarray(char_vectors.astype(BF16)),
        "wconv": np.ascontiguousarray(wconv.astype(BF16)),
        "wproj": np.ascontiguousarray(wproj.astype(BF16)),
        "whw": np.ascontiguousarray(whw.astype(BF16)),
        "bias": np.ascontiguousarray(bias),
    }


# ---------------------------------------------------------------- program
def build_program(n_tiles=N_TILES, repeat=1):  # n_tiles: prefix of TILES
    from concourse import bacc
    import concourse.mybir as mybir
    from concourse.tile import TileContext

    dt = mybir.dt
    AF = mybir.ActivationFunctionType
    AL = mybir.AluOpType
    units, sched = build_schedule()
    U = len(units)

    nc = bacc.Bacc("TRN2", target_bir_lowering=False, debug=False, num_devices=N_CORES)

    idx_d = nc.dram_tensor("idx", [16, TOK_PER_CORE], dt.int16, kind="ExternalInput")
    cv_d = nc.dram_tensor("cv", [VOCAB, D], dt.bfloat16, kind="ExternalInput")
    wconv_d = nc.dram_tensor("wconv", [128, U, 128], dt.bfloat16, kind="ExternalInput")
    wproj_d = nc.dram_tensor("wproj", [128, 4, 128], dt.bfloat16, kind="ExternalInput")
    whw_d = nc.dram_tensor("whw", [128, 4, 128], dt.bfloat16, kind="ExternalInput")
    bias_d = nc.dram_tensor("bias", [128, 4], dt.float32, kind="ExternalInput")
    out_d = nc.dram_tensor("out", [128, TOK_PER_CORE], dt.float32, kind="ExternalOutput")

    with TileContext(nc) as tc:
        with (
            tc.tile_pool(name="const", bufs=1) as cpool,
            tc.tile_pool(name="io", bufs=2) as iopool,
            tc.tile_pool(name="work", bufs=2) as wpool,
            tc.tile_pool(name="merge", bufs=8) as mpool,
            tc.tile_pool(name="pw", bufs=3) as pwpool,
            tc.tile_pool(name="grp_psum", bufs=2, space="PSUM") as gpsum,
            tc.tile_pool(name="vec_psum", bufs=2, space="PSUM") as vpsum,
        ):
            cv_sb = cpool.tile([VOCAB, D], dt.bfloat16)
            nc.sync.dma_start(cv_sb, cv_d.ap())
            wconv_sb = cpool.tile([128, U, 128], dt.bfloat16)
            nc.sync.dma_start(wconv_sb, wconv_d.ap())
            wproj_sb = cpool.tile([128, 4, 128], dt.bfloat16)
            nc.sync.dma_start(wproj_sb, wproj_d.ap())
            whw_sb = cpool.tile([128, 4, 128], dt.bfloat16)
            nc.sync.dma_start(whw_sb, whw_d.ap())
            bias_sb = cpool.tile([128, 4], dt.float32)
            nc.sync.dma_start(bias_sb, bias_d.ap())
            iota_i = cpool.tile([VOCAB, 1], dt.int32)
            nc.gpsimd.iota(iota_i, pattern=[[1, 1]], base=0, channel_multiplier=1)
            iota_sb = cpool.tile([VOCAB, 1], dt.float32)
            nc.vector.tensor_copy(iota_sb, iota_i)

            for _rep in range(repeat):
                for ti in range(n_tiles):
                    tok0, Tc = TILES[ti]
                    # ---- one-hot
                    idx_b = iopool.tile([VOCAB, 16, Tc], dt.int16, tag="idxb")
                    nc.sync.dma_start(
                        idx_b, idx_d.ap()[:, tok0:tok0 + Tc].partition_broadcast(VOCAB)
                    )
                    oh = iopool.tile([VOCAB, 16, Tc], dt.bfloat16, tag="oh")
                    nc.vector.tensor_scalar(oh, idx_b, iota_sb[:, :], None, AL.is_equal)

                    # ---- embeddings, paired layout
                    e_sb = iopool.tile([128, 8, Tc], dt.bfloat16, tag="esb")
                    for r0 in range(0, 8, GRP):
                        n = min(GRP, 8 - r0)
                        g = gpsum.tile([128, GRP, 512], dt.float32, tag="grp")
                        for i in range(n):
                            p = r0 + i
                            nc.tensor.matmul(g[0:64, i, 0:Tc], cv_sb, oh[:, 2 * p, :],
                                             start=True, stop=True, tile_position=(0, 0))
                            nc.tensor.matmul(g[64:128, i, 0:Tc], cv_sb, oh[:, 2 * p + 1, :],
                                             start=True, stop=True, tile_position=(0, 64))
                        nc.scalar.copy(e_sb[:, r0:r0 + n, :], g[:, 0:n, 0:Tc])

                    # ---- conv + split max-pool
                    mfin = wpool.tile([128, 4, Tc], dt.bfloat16, tag="m4")
                    for wi, (w, wl) in enumerate(sched):
                        L = len(wl)
                        groups = window_groups(L)
                        def is_act(gi):
                            # alternate ACT/DVE; windows 4&5 give ACT one extra
                            # group to balance DVE (the busiest engine)
                            return gi % 2 == 0 or (wi >= 2 and gi == len(groups) - 2)
                        na = sum(n for gi, (l0, n) in enumerate(groups) if is_act(gi))
                        pw = pwpool.tile([128, max(na, 1), Tc], dt.bfloat16, tag="pw")
                        items = []  # (kind, ap) partial maxes / positions
                        a_fill = 0
                        for gi, (l0, n) in enumerate(groups):
                            g = gpsum.tile([128, GRP, 512], dt.float32, tag="grp")
                            for li in range(n):
                                ops = wl[l0 + li]
                                for oi, (u, pair) in enumerate(ops):
                                    nc.tensor.matmul(
                                        g[:, li, 0:Tc], wconv_sb[:, u, :],
                                        e_sb[:, pair, :],
                                        start=(oi == 0), stop=(oi == len(ops) - 1),
                                    )
                            if is_act(gi):  # ACT extract positions
                                nc.scalar.copy(pw[:, a_fill:a_fill + n, :], g[:, 0:n, 0:Tc])
                                for i in range(n):
                                    items.append(pw[:, a_fill + i, :])
                                a_fill += n
                            else:  # DVE grouped max straight from PSUM
                                pd = mpool.tile([128, Tc], dt.bfloat16, tag="pd")
                                if n == 1:
                                    nc.vector.tensor_copy(pd, g[:, 0, 0:Tc])
                                else:
                                    nc.vector.tensor_reduce(
                                        pd,
                                        g[:, 0:n, 0:Tc].rearrange("p g t -> p t g"),
                                        axis=mybir.AxisListType.X,
                                        op=AL.max,
                                    )
                                items.append(pd)
                        # bf16 max tree on DVE (2x_1p); final op writes
                        # mfin[:, wi, :]. (GPSIMD can't take TensorTensor --
                        # walrus rejects the Pool engine for it.)
                        while len(items) > 2:
                            a = items.pop(0)
                            b = items.pop(0)
                            t2 = mpool.tile([128, Tc], dt.bfloat16, tag="tmerge")
                            nc.vector.tensor_max(t2, a, b)
                            items.append(t2)
                        nc.vector.tensor_max(mfin[:, wi, :], items[0], items[1])

                    th = wpool.tile([128, 4, Tc], dt.bfloat16, tag="th")
                    nc.scalar.activation(th[:, :, :], mfin[:, :, :], AF.Tanh)

                    # ---- projection
                    x_ps = vpsum.tile([128, 512], dt.float32, tag="vec")
                    for c in range(4):
                        nc.tensor.matmul(x_ps[:, 0:Tc], wproj_sb[:, c, :], th[:, c, :],
                                         start=(c == 0), stop=(c == 3))
                    xs = wpool.tile([128, Tc], dt.bfloat16, tag="xs")
                    nc.scalar.copy(xs, x_ps[:, 0:Tc])

                    # ---- highway x2
                    for hl in range(2):
                        t_ps = vpsum.tile([128, 512], dt.float32, tag="vec")
                        g_ps = vpsum.tile([128, 512], dt.float32, tag="vec")
                        nc.tensor.matmul(t_ps[:, 0:Tc], whw_sb[:, 2 * hl, :], xs,
                                         start=True, stop=True)
                        nc.tensor.matmul(g_ps[:, 0:Tc], whw_sb[:, 2 * hl + 1, :], xs,
                                         start=True, stop=True)
                        tt = wpool.tile([128, Tc], dt.bfloat16, tag="tt")
                        gg = wpool.tile([128, Tc], dt.bfloat16, tag="gg")
                        nc.scalar.activation(tt, t_ps[:, 0:Tc], AF.Relu,
                                             bias=bias_sb[:, 2 * hl:2 * hl + 1], scale=1.0)
                        nc.scalar.activation(gg, g_ps[:, 0:Tc], AF.Sigmoid,
                                             bias=bias_sb[:, 2 * hl + 1:2 * hl + 2], scale=1.0)
                        dd = wpool.tile([128, Tc], dt.bfloat16, tag="dd")
                        gd = wpool.tile([128, Tc], dt.bfloat16, tag="gd")
                        nc.vector.tensor_sub(dd, tt, xs)
                        nc.vector.tensor_mul(gd, gg, dd)
                        if hl == 0:
                            xs2 = wpool.tile([128, Tc], dt.bfloat16, tag="xs")
                            nc.vector.tensor_add(xs2, xs, gd)
                            xs = xs2
                        else:
                            xf = wpool.tile([128, Tc], dt.float32, tag="xf")
                            nc.vector.tensor_add(xf, xs, gd)
                            nc.sync.dma_start(out_d.ap()[:, tok0:tok0 + Tc], xf)

    nc.compile()
    return nc


# ---------------------------------------------------------------- runner
def _make_sharded(nc):
    import jax
    from jax.sharding import Mesh, PartitionSpec
    from jax.experimental.shard_map import shard_map
    from concourse import bass2jax, mybir

    bass2jax.install_neuronx_cc_hook()
    partition_name = nc.partition_id_tensor.name if nc.partition_id_tensor else None
    in_names, out_names, out_avals = [], [], []
    for alloc in nc.m.functions[0].allocations:
        if not isinstance(alloc, mybir.MemoryLocationSet):
            continue
        name = alloc.memorylocations[0].name
        if alloc.kind == "ExternalInput":
            if name != partition_name:
                in_names.append(name)
        elif alloc.kind == "ExternalOutput":
            out_names.append(name)
            out_avals.append(
                jax.core.ShapedArray(tuple(alloc.tensor_shape), mybir.dt.np(alloc.dtype))
            )
    n_params = len(in_names)
    all_in_names = in_names + out_names
    if partition_name is not None:
        all_in_names = all_in_names + [partition_name]

    def _body(*args):
        operands = list(args)
        if partition_name is not None:
            operands.append(bass2jax.partition_id_tensor())
        outs = bass2jax._bass_exec_p.bind(
            *operands,
            out_avals=tuple(out_avals),
            in_names=tuple(all_in_names),
            out_names=tuple(out_names),
            lowering_input_output_aliases=(),
            sim_require_finite=True,
            sim_require_nnan=True,
            nc=nc,
        )
        return tuple(outs)

    devices = jax.devices()[:N_CORES]
    mesh = Mesh(np.asarray(devices), ("core",))
    n_outs = len(out_names)
    in_specs = (PartitionSpec("core"),) * (n_params + n_outs)
    out_specs = (PartitionSpec("core"),) * n_outs
    fn = jax.jit(
        shard_map(_body, mesh=mesh, in_specs=in_specs, out_specs=out_specs,
                  check_rep=False),
        keep_unused=True,
    )
    meta = {"in_names": in_names, "out_names": out_names, "out_avals": out_avals,
            "n_params": n_params}
    return fn, meta


def _get_runner():
    if "runner" not in _cache:
        nc = build_program()
        _cache["nc"] = nc
        _cache["runner"] = _make_sharded(nc)
    return _cache["runner"]


def _concat_inputs(in_maps, meta):
    concat_in = [
        np.concatenate([in_maps[c][name] for c in range(N_CORES)], axis=0)
        for name in meta["in_names"]
    ]
    concat_zeros = [
        np.zeros((N_CORES * a.shape[0], *a.shape[1:]), a.dtype)
        for a in meta["out_avals"]
    ]
    return concat_in, concat_zeros


def make_in_maps(char_idxs, char_vectors, filt2, filt3, filt4, filt5, w_proj,
                 t_w0, t_b0, t_w1, t_b1, g_w0, g_b0, g_w1, g_b1):
    wts = prep_weights(
        np.asarray(char_vectors, np.float32),
        {2: np.asarray(filt2, np.float32), 3: np.asarray(filt3, np.float32),
         4: np.asarray(filt4, np.float32), 5: np.asarray(filt5, np.float32)},
        np.asarray(w_proj, np.float32),
        [np.asarray(t_w0, np.float32), np.asarray(g_w0, np.float32),
         np.asarray(t_w1, np.float32), np.asarray(g_w1, np.float32)],
        [np.asarray(t_b0, np.float32), np.asarray(g_b0, np.float32),
         np.asarray(t_b1, np.float32), np.asarray(g_b1, np.float32)],
    )
    idx = np.asarray(char_idxs)
    assert idx.shape == (B, S, W)
    rows_per_core = B // N_CORES
    in_maps = []
    for c in range(N_CORES):
        m = dict(wts)
        m["idx"] = np.ascontiguousarray(
            idx[c * rows_per_core:(c + 1) * rows_per_core]
            .reshape(TOK_PER_CORE, 16).T.astype(np.int16)
        )
        in_maps.append(m)
    return in_maps


def kernel(**inputs) -> np.ndarray:
    in_maps = make_in_maps(**inputs)
    sharded, meta = _get_runner()
    concat_in, concat_zeros = _concat_inputs(in_maps, meta)
    out_arrs = sharded(*concat_in, *concat_zeros)
    out = np.asarray(out_arrs[0])  # [8*128, 3200]
    rows_per_core = B // N_CORES
    parts = []
    for c in range(N_CORES):
        oc = out[c * 128:(c + 1) * 128]  # [128, 3200]
        parts.append(oc.T.reshape(rows_per_core, S, HID))
    return np.ascontiguousarray(np.concatenate(parts, axis=0))


def time_kernel(inputs, repeat=(8, 25), reps=20):
    """Per-pass exec time from the slope between two repeat factors.

    Wall(R) = dispatch + hidden-overlap + R * exec, so
    exec = (wall(R2) - wall(R1)) / (R2 - R1) with interleaved sampling.
    """
    import time
    import jax
    from jax.sharding import Mesh, PartitionSpec, NamedSharding

    in_maps = make_in_maps(**inputs)
    sharded, meta = _get_runner()
    concat_in, concat_zeros = _concat_inputs(in_maps, meta)
    mesh = Mesh(np.asarray(jax.devices()[:N_CORES]), ("core",))
    shd = NamedSharding(mesh, PartitionSpec("core"))
    d_in = [jax.device_put(a, shd) for a in concat_in]
    d_zero = [jax.device_put(a, shd) for a in concat_zeros]

    r1, r2 = repeat
    fns = []
    for r in (r1, r2):
        key = ("rep", r)
        if key not in _cache:
            nc_r = build_program(repeat=r)
            _cache[key] = _make_sharded(nc_r)
        fns.append(_cache[key][0])
    fn_1, fn_2 = fns

    def timed(fn, args):
        t0 = time.perf_counter()
        out = fn(*args)
        jax.block_until_ready(out)
        return time.perf_counter() - t0

    timed(fn_1, (*d_in, *d_zero))
    timed(fn_2, (*d_in, *d_zero))
    diffs, t1s = [], []
    for _ in range(reps):
        a = timed(fn_1, (*d_in, *d_zero))
        b = timed(fn_2, (*d_in, *d_zero))
        t1s.append(a)
        diffs.append(b - a)
    diffs.sort()
    t1s.sort()
    med = diffs[len(diffs) // 2]
    per_pass = med / (r2 - r1)
    return per_pass * 1e9, t1s[len(t1s) // 2] * 1e9, med * 1e9
